# revision 33
# baseline (speedup 1.0000x reference)
"""Trainium2 Bass kernel for the Antenna message-generation MLP.

Reference computation (per batch b, RF-chain r, antenna u):
    x[b,r,u,:48] = concat(F[b,:,r], sum_u C[b,u,r,:], H[b,u,8r:8r+8], H[b,u,64+8r:64+8r+8])
    out[b,r,u,:] = tanh(relu(relu(x@W1+b1)@W2+b2)@W3+b3)

Strategy: pure data parallelism over the batch dim across 8 NeuronCores
(256 batches/core).  On each core the 16384 rows are processed in 32
tiles of 512 rows; activations are kept feature-on-partition so the
three matmul layers chain without transposes, all matmuls in float32r
(full-rate fp32).  The input gather (F transpose/broadcast, C u-sum,
H real/imag split) is built on-chip into a 64-partition X^T whose
layout folds the L1 bias in via a constant ones row:
    [0:16]=F  [16]=ones  [17:32]=0  [32:48]=c  [48:64]=0  [64:80]=h
with W1 zero-padded to match.  PE pair-packed transposes handle the
partition moves; DMA handles non-32-aligned partition relocation.
"""

import sys
import types

import numpy as np

# This image's `antenv` lacks `axon_hooks`; bass_utils imports it when
# BASS_TRACE is set.  Register a no-op stand-in so tracing degrades
# gracefully instead of crashing (real hook installed by test harness).
try:
    import antenv.axon_hooks  # noqa: F401
except ImportError:
    import antenv

    _m = types.ModuleType("antenv.axon_hooks")
    _m._hook = None
    _m.set_axon_ntff_profile_hook = lambda h: setattr(_m, "_hook", h)
    _m.get_axon_ntff_profile_hook = lambda: _m._hook
    sys.modules["antenv.axon_hooks"] = _m
    antenv.axon_hooks = _m

import concourse.bacc as bacc
import concourse.mybir as mybir
import concourse.tile as tile
from concourse.bass_utils import run_bass_kernel_spmd

F32 = mybir.dt.float32
F32R = mybir.dt.float32r

N_CORES = 8
B_FULL = 2048
B_SH = B_FULL // N_CORES    # 256 batches per core
U = 8
R = 8
M = 16
FDIM = 16
H1 = 512
H2 = 512

BG = 16                     # batches per build chunk
G = B_SH // BG              # 16 chunks per core
ROWS_CHUNK = BG * R * U     # 1024 rows per chunk
TILE = 512                  # rows per MLP tile (one PSUM bank of fp32)
XP = 80                     # X^T partitions (padded layout)

_CACHE = {}


def _build():
    nc = bacc.Bacc("TRN2", target_bir_lowering=False, debug=False)

    C_ext = nc.dram_tensor("C", [B_SH, U, R, M], F32, kind="ExternalInput")
    F_ext = nc.dram_tensor("F", [B_SH, FDIM, R], F32, kind="ExternalInput")
    H_ext = nc.dram_tensor("H", [B_SH, U, 2 * 64], F32, kind="ExternalInput")
    W1_ext = nc.dram_tensor("W1", [48, H1], F32, kind="ExternalInput")
    b1_ext = nc.dram_tensor("b1", [H1], F32, kind="ExternalInput")
    W2_ext = nc.dram_tensor("W2", [H1, H2], F32, kind="ExternalInput")
    b2_ext = nc.dram_tensor("b2", [H2], F32, kind="ExternalInput")
    W3_ext = nc.dram_tensor("W3", [H2, M], F32, kind="ExternalInput")
    b3_ext = nc.dram_tensor("b3", [M], F32, kind="ExternalInput")
    eye_ext = nc.dram_tensor("eye128", [128, 128], F32, kind="ExternalInput")
    # xinit row 0 is ones (the folded-bias row), rest zeros
    xinit_ext = nc.dram_tensor("xinit", [32, ROWS_CHUNK], F32, kind="ExternalInput")
    out_ext = nc.dram_tensor("out", [B_SH, R, U, M], F32, kind="ExternalOutput")

    out_rows = out_ext.ap().rearrange("b r u m -> (b r u) m")  # [16384, 16]

    relu = mybir.ActivationFunctionType.Relu
    tanh = mybir.ActivationFunctionType.Tanh
    axis_x = mybir.AxisListType.X
    op_add = mybir.AluOpType.add

    with tile.TileContext(nc) as tc:
        with (
            tc.tile_pool(name="consts", bufs=1) as consts,
            tc.tile_pool(name="loads", bufs=3) as loads,
            tc.tile_pool(name="acts", bufs=2) as acts,
            tc.tile_pool(name="outs", bufs=3) as outs,
            tc.tile_pool(name="p1", bufs=3, space="PSUM") as p1p,
            tc.tile_pool(name="p2", bufs=3, space="PSUM") as p2p,
            tc.tile_pool(name="psm", bufs=2, space="PSUM") as psm,
        ):
            # ---- constants -------------------------------------------------
            eye = consts.tile([128, 128], F32)
            nc.sync.dma_start(eye[:].bitcast(F32R), eye_ext.ap().bitcast(F32R))
            # W1 padded to the X^T layout, bias folded in as row 16
            w1raw = consts.tile([XP, H1], F32)
            nc.gpsimd.memset(w1raw[:], 0.0)
            nc.sync.dma_start(w1raw[0:16, :], W1_ext[0:16])
            nc.sync.dma_start(
                w1raw[16:17, :], b1_ext.ap().rearrange("(o n) -> o n", o=1)
            )
            nc.sync.dma_start(w1raw[32:48, :], W1_ext[16:32])
            nc.sync.dma_start(w1raw[64:80, :], W1_ext[32:48])
            w1 = consts.tile([XP, H1], F32)
            nc.vector.tensor_copy(w1[:].bitcast(F32R), w1raw[:])
            w2 = consts.tile([128, 4, H2], F32)
            nc.sync.dma_start(
                w2[:].bitcast(F32R),
                W2_ext.ap().rearrange("(s p) n -> p s n", p=128).bitcast(F32R),
            )
            w3raw = consts.tile([128, 4, 2 * M], F32)
            nc.gpsimd.memset(w3raw[:], 0.0)
            nc.sync.dma_start(
                w3raw[:, :, 0:M],
                W3_ext.ap().rearrange("(s p) m -> p s m", p=128),
            )
            w3 = consts.tile([128, 4, 2 * M], F32)
            nc.vector.tensor_copy(w3[:].bitcast(F32R), w3raw[:])
            b2 = consts.tile([128, 4], F32)
            nc.sync.dma_start(b2[:], b2_ext.ap().rearrange("(s p) -> p s", p=128))
            zero_bc = consts.tile([128, 1], F32)
            nc.gpsimd.memset(zero_bc[:], 0.0)
            b3 = consts.tile([2 * M, 1], F32)
            nc.gpsimd.memset(b3[:], 0.0)
            nc.sync.dma_start(b3[0:M, :], b3_ext.ap().rearrange("(m o) -> m o", o=1))
            # F transposed once for the whole core: [16 x b x r]
            ft = consts.tile([FDIM, B_SH, R], F32)
            nc.sync.dma_start(ft[:], F_ext.ap().rearrange("b f r -> f b r"))

            # Persistent double-buffered X^T; one-time init of the ones/zero
            # band [16:32] (DMA may write any partition base).
            xts = []
            for i in range(2):
                xt = consts.tile([XP, ROWS_CHUNK], F32, tag=f"xt{i}")
                nc.sync.dma_start(
                    xt[16:32, :].bitcast(F32R), xinit_ext[0:16].bitcast(F32R)
                )
                nc.sync.dma_start(
                    xt[48:64, :].bitcast(F32R), xinit_ext[16:32].bitcast(F32R)
                )
                xts.append(xt)

            for g in range(G):
                b0 = g * BG
                xt = xts[g % 2]
                # ---- load chunk into r-padded 32-col blocks ---------------
                # c_pad [128 x 256]: cols r*32 + (m | 16 pad)
                # h_pad [128 x 256]: cols r*32 + (i*8+k | 16 pad)
                c_pad = loads.tile([BG * U, 256], F32, tag="c_pad")
                nc.gpsimd.memset(c_pad[:], 0.0)
                cp_v = c_pad[:].rearrange("p (r w) -> p r w", r=R)
                nc.sync.dma_start(
                    cp_v[:, :, 0:M],
                    C_ext[b0 : b0 + BG].rearrange("b u r m -> (b u) r m"),
                )
                h_pad = loads.tile([BG * U, 256], F32, tag="h_pad")
                nc.gpsimd.memset(h_pad[:], 0.0)
                hp_v = h_pad[:].rearrange("p (r w) -> p r w", r=R)
                h_src = H_ext[b0 : b0 + BG].rearrange(
                    "b u (i r k) -> (b u) i r k", i=2, r=R
                )
                for i in range(2):
                    nc.sync.dma_start(hp_v[:, :, 8 * i : 8 * i + 8], h_src[:, i])

                # ---- DVE 32x32 stream transposes --------------------------
                # cT[32B+m, r*32 + b4*8 + u] = C[4B+b4, u, r, m]   (m < 16)
                # hT[32B+f, r*32 + b4*8 + u] = H-feat f of (4B+b4, u, r)
                cT = loads.tile([BG * U, 256], F32, tag="cT")
                nc.vector.transpose(cT[:], c_pad[:])
                hT = loads.tile([BG * U, 256], F32, tag="hT")
                nc.vector.transpose(hT[:], h_pad[:])

                # ---- C path: u-sum then broadcast over u ------------------
                c_red = loads.tile([BG * U, 32], F32, tag="c_red")
                nc.vector.tensor_reduce(
                    c_red[:],
                    cT[:].rearrange("p (rb u) -> p rb u", u=U),
                    axis_x, op_add,
                )
                # c_red[32B+m, r*4 + b4] = c[4B+b4, r, m]
                xt_c = xt[32:48, :].rearrange(
                    "p (B b4 r u) -> p B r b4 u", B=4, b4=4, u=U
                )
                for B in range(4):
                    nc.vector.tensor_copy(
                        xt_c[:, B].bitcast(F32R),
                        c_red[32 * B : 32 * B + 16, :]
                        .rearrange("p (r b4) -> p r b4", b4=4)
                        .unsqueeze(3)
                        .broadcast_to((16, R, 4, U)),
                    )

                # ---- H path: per-B-band copies into xt --------------------
                xt_h = xt[64:80, :].rearrange(
                    "p (B b4 r u) -> p B r b4 u", B=4, b4=4, u=U
                )
                for B in range(4):
                    nc.vector.tensor_copy(
                        xt_h[:, B].bitcast(F32R),
                        hT[32 * B : 32 * B + 16, :].rearrange(
                            "p (r b4 u) -> p r b4 u", b4=4, u=U
                        ),
                    )

                # ---- F broadcast over u -----------------------------------
                nc.scalar.copy(
                    xt[0:16, :]
                    .rearrange("p (b r u) -> p b r u", r=R, u=U)
                    .bitcast(F32R),
                    ft[:, b0 : b0 + BG, :]
                    .unsqueeze(3)
                    .broadcast_to((FDIM, BG, R, U)),
                )

                # ---- MLP over two 512-row halves --------------------------
                for h in range(2):
                    cols = slice(h * TILE, (h + 1) * TILE)
                    xin = xt[:, cols].bitcast(F32R)

                    a1s = []
                    for s in range(4):
                        ps1 = p1p.tile([128, TILE], F32, tag="ps1")
                        nc.tensor.matmul(
                            ps1[:],
                            w1[:, s * 128 : (s + 1) * 128].bitcast(F32R),
                            xin,
                            start=True, stop=True,
                        )
                        a1_s = acts.tile([128, TILE], F32, tag=f"a1{s}")
                        # L1 bias folded in via the ones row -> plain relu
                        if s < 2:
                            nc.scalar.activation(
                                a1_s[:].bitcast(F32R), ps1[:], relu
                            )
                        else:
                            nc.vector.tensor_scalar_max(
                                a1_s[:].bitcast(F32R), ps1[:], 0.0
                            )
                        a1s.append(a1_s)

                    a2s = []
                    for t in range(4):
                        ps2 = p2p.tile([128, TILE], F32, tag="ps2")
                        for s in range(4):
                            nc.tensor.matmul(
                                ps2[:],
                                w2[:, s, t * 128 : (t + 1) * 128].bitcast(F32R),
                                a1s[s][:].bitcast(F32R),
                                start=(s == 0), stop=(s == 3),
                            )
                        a2_t = acts.tile([128, TILE], F32, tag=f"a2{t}")
                        nc.scalar.activation(
                            a2_t[:].bitcast(F32R), ps2[:], relu,
                            bias=b2[:, t : t + 1],
                        )
                        a2s.append(a2_t)

                    ps3 = psm.tile([2 * M, TILE], F32, tag="sm")
                    for s in range(4):
                        nc.tensor.matmul(
                            ps3[:],
                            w3[:, s, :].bitcast(F32R),
                            a2s[s][:].bitcast(F32R),
                            start=(s == 0), stop=(s == 3),
                        )
                    yt = outs.tile([2 * M, TILE], F32, tag="yt")
                    nc.scalar.activation(yt[:], ps3[:], tanh, bias=b3[:, 0:1])

                    # 32x32 stream transpose -> [32 x (16 rowblk, 16 m | 16 pad)]
                    o_t = outs.tile([2 * M, TILE], F32, tag="o_t")
                    nc.vector.transpose(o_t[:], yt[:])
                    row0 = (2 * g + h) * TILE
                    nc.sync.dma_start(
                        out_rows[row0 : row0 + TILE].rearrange(
                            "(q p) m -> p q m", p=32
                        ),
                        o_t[:].rearrange("p (q w) -> p q w", w=32)[:, :, 0:M],
                    )

    nc.compile()
    return nc


def _get_nc():
    if "nc" not in _CACHE:
        _CACHE["nc"] = _build()
    return _CACHE["nc"]


def _xinit():
    x = np.zeros((32, ROWS_CHUNK), dtype=np.float32)
    x[0, :] = 1.0
    return x


def run(inputs, trace=False):
    nc = _get_nc()
    np_in = {k: np.ascontiguousarray(np.asarray(v, dtype=np.float32))
             for k, v in inputs.items()}
    eye = np.eye(128, dtype=np.float32)
    xinit = _xinit()
    in_maps = []
    for i in range(N_CORES):
        sl = slice(i * B_SH, (i + 1) * B_SH)
        in_maps.append({
            "C": np_in["C"][sl],
            "F": np_in["F"][sl],
            "H": np_in["H"][sl],
            "W1": np_in["W1"], "b1": np_in["b1"],
            "W2": np_in["W2"], "b2": np_in["b2"],
            "W3": np_in["W3"], "b3": np_in["b3"],
            "eye128": eye,
            "xinit": xinit,
        })
    res = run_bass_kernel_spmd(nc, in_maps, list(range(N_CORES)), trace=trace)
    out = np.concatenate([res.results[i]["out"] for i in range(N_CORES)], axis=0)
    return out, res


def kernel(**inputs):
    out, _ = run(inputs, trace=False)
    return out


# revision 34
# speedup vs baseline: 1.0977x; 1.0977x over previous
"""Trainium2 Bass kernel for the Antenna message-generation MLP.

Reference computation (per batch b, RF-chain r, antenna u):
    x[b,r,u,:48] = concat(F[b,:,r], sum_u C[b,u,r,:], H[b,u,8r:8r+8], H[b,u,64+8r:64+8r+8])
    out[b,r,u,:] = tanh(relu(relu(x@W1+b1)@W2+b2)@W3+b3)

Strategy: pure data parallelism over the batch dim across 8 NeuronCores
(256 batches/core).  On each core the 16384 rows are processed in 32
tiles of 512 rows; activations are kept feature-on-partition so the
three matmul layers chain without transposes, all matmuls in float32r
(full-rate fp32).  The input gather (F transpose/broadcast, C u-sum,
H real/imag split) is built on-chip into a 64-partition X^T whose
layout folds the L1 bias in via a constant ones row:
    [0:16]=F  [16]=ones  [17:32]=0  [32:48]=c  [48:64]=0  [64:80]=h
with W1 zero-padded to match.  PE pair-packed transposes handle the
partition moves; DMA handles non-32-aligned partition relocation.
"""

import sys
import types

import numpy as np

# This image's `antenv` lacks `axon_hooks`; bass_utils imports it when
# BASS_TRACE is set.  Register a no-op stand-in so tracing degrades
# gracefully instead of crashing (real hook installed by test harness).
try:
    import antenv.axon_hooks  # noqa: F401
except ImportError:
    import antenv

    _m = types.ModuleType("antenv.axon_hooks")
    _m._hook = None
    _m.set_axon_ntff_profile_hook = lambda h: setattr(_m, "_hook", h)
    _m.get_axon_ntff_profile_hook = lambda: _m._hook
    sys.modules["antenv.axon_hooks"] = _m
    antenv.axon_hooks = _m

import concourse.bacc as bacc
import concourse.mybir as mybir
import concourse.tile as tile
from concourse.bass_utils import run_bass_kernel_spmd

F32 = mybir.dt.float32
F32R = mybir.dt.float32r

N_CORES = 8
B_FULL = 2048
B_SH = B_FULL // N_CORES    # 256 batches per core
U = 8
R = 8
M = 16
FDIM = 16
H1 = 512
H2 = 512

BG = 16                     # batches per build chunk
G = B_SH // BG              # 16 chunks per core
ROWS_CHUNK = BG * R * U     # 1024 rows per chunk
TILE = 512                  # rows per MLP tile (one PSUM bank of fp32)
XP = 80                     # X^T partitions (padded layout)

_CACHE = {}


def _build():
    nc = bacc.Bacc("TRN2", target_bir_lowering=False, debug=False)

    C_ext = nc.dram_tensor("C", [B_SH, U, R, M], F32, kind="ExternalInput")
    F_ext = nc.dram_tensor("F", [B_SH, FDIM, R], F32, kind="ExternalInput")
    H_ext = nc.dram_tensor("H", [B_SH, U, 2 * 64], F32, kind="ExternalInput")
    W1_ext = nc.dram_tensor("W1", [48, H1], F32, kind="ExternalInput")
    b1_ext = nc.dram_tensor("b1", [H1], F32, kind="ExternalInput")
    W2_ext = nc.dram_tensor("W2", [H1, H2], F32, kind="ExternalInput")
    b2_ext = nc.dram_tensor("b2", [H2], F32, kind="ExternalInput")
    W3_ext = nc.dram_tensor("W3", [H2, M], F32, kind="ExternalInput")
    b3_ext = nc.dram_tensor("b3", [M], F32, kind="ExternalInput")
    eye_ext = nc.dram_tensor("eye128", [128, 128], F32, kind="ExternalInput")
    # xinit row 0 is ones (the folded-bias row), rest zeros
    xinit_ext = nc.dram_tensor("xinit", [32, ROWS_CHUNK], F32, kind="ExternalInput")
    out_ext = nc.dram_tensor("out", [B_SH, R, U, M], F32, kind="ExternalOutput")

    out_rows = out_ext.ap().rearrange("b r u m -> (b r u) m")  # [16384, 16]

    relu = mybir.ActivationFunctionType.Relu
    tanh = mybir.ActivationFunctionType.Tanh
    axis_x = mybir.AxisListType.X
    op_add = mybir.AluOpType.add

    with tile.TileContext(nc) as tc:
        with (
            tc.tile_pool(name="consts", bufs=1) as consts,
            tc.tile_pool(name="loads", bufs=3) as loads,
            tc.tile_pool(name="acts", bufs=3) as acts,
            tc.tile_pool(name="outs", bufs=3) as outs,
            tc.tile_pool(name="p1", bufs=3, space="PSUM") as p1p,
            tc.tile_pool(name="p2", bufs=3, space="PSUM") as p2p,
            tc.tile_pool(name="psm", bufs=2, space="PSUM") as psm,
        ):
            # ---- constants -------------------------------------------------
            eye = consts.tile([128, 128], F32)
            nc.sync.dma_start(eye[:].bitcast(F32R), eye_ext.ap().bitcast(F32R))
            # W1 padded to the X^T layout, bias folded in as row 16
            w1raw = consts.tile([XP, H1], F32)
            nc.gpsimd.memset(w1raw[:], 0.0)
            nc.sync.dma_start(w1raw[0:16, :], W1_ext[0:16])
            nc.sync.dma_start(
                w1raw[16:17, :], b1_ext.ap().rearrange("(o n) -> o n", o=1)
            )
            nc.sync.dma_start(w1raw[32:48, :], W1_ext[16:32])
            nc.sync.dma_start(w1raw[64:80, :], W1_ext[32:48])
            w1 = consts.tile([XP, H1], F32)
            nc.vector.tensor_copy(w1[:].bitcast(F32R), w1raw[:])
            w2 = consts.tile([128, 4, H2], F32)
            nc.sync.dma_start(
                w2[:].bitcast(F32R),
                W2_ext.ap().rearrange("(s p) n -> p s n", p=128).bitcast(F32R),
            )
            w3 = consts.tile([128, 4, M], F32)
            nc.sync.dma_start(
                w3[:].bitcast(F32R),
                W3_ext.ap().rearrange("(s p) m -> p s m", p=128).bitcast(F32R),
            )
            b2 = consts.tile([128, 4], F32)
            nc.sync.dma_start(b2[:], b2_ext.ap().rearrange("(s p) -> p s", p=128))
            zero_bc = consts.tile([128, 1], F32)
            nc.gpsimd.memset(zero_bc[:], 0.0)
            b3 = consts.tile([M, 1], F32)
            nc.sync.dma_start(b3[:], b3_ext.ap().rearrange("(m o) -> m o", o=1))
            # F transposed once for the whole core: [16 x b x r]
            ft = consts.tile([FDIM, B_SH, R], F32)
            nc.sync.dma_start(ft[:], F_ext.ap().rearrange("b f r -> f b r"))

            # Persistent double-buffered X^T; one-time init of the ones/zero
            # band [16:32] (DMA may write any partition base).
            xts = []
            for i in range(2):
                xt = consts.tile([XP, ROWS_CHUNK], F32, tag=f"xt{i}")
                nc.sync.dma_start(
                    xt[16:32, :].bitcast(F32R), xinit_ext[0:16].bitcast(F32R)
                )
                nc.sync.dma_start(
                    xt[48:64, :].bitcast(F32R), xinit_ext[16:32].bitcast(F32R)
                )
                xts.append(xt)

            for g in range(G):
                b0 = g * BG
                xt = xts[g % 2]
                # ---- load chunk into r-padded 32-col blocks ---------------
                # c_pad [128 x 256]: cols r*32 + (m | 16 pad)
                # h_pad [128 x 256]: cols r*32 + (i*8+k | 16 pad)
                c_pad = loads.tile([BG * U, 256], F32, tag="c_pad")
                nc.gpsimd.memset(c_pad[:], 0.0)
                cp_v = c_pad[:].rearrange("p (r w) -> p r w", r=R)
                nc.sync.dma_start(
                    cp_v[:, :, 0:M],
                    C_ext[b0 : b0 + BG].rearrange("b u r m -> (b u) r m"),
                )
                h_pad = loads.tile([BG * U, 256], F32, tag="h_pad")
                nc.gpsimd.memset(h_pad[:], 0.0)
                hp_v = h_pad[:].rearrange("p (r w) -> p r w", r=R)
                h_src = H_ext[b0 : b0 + BG].rearrange(
                    "b u (i r k) -> (b u) i r k", i=2, r=R
                )
                for i in range(2):
                    nc.sync.dma_start(hp_v[:, :, 8 * i : 8 * i + 8], h_src[:, i])

                # ---- DVE 32x32 stream transposes --------------------------
                # cT[32B+m, r*32 + b4*8 + u] = C[4B+b4, u, r, m]   (m < 16)
                # hT[32B+f, r*32 + b4*8 + u] = H-feat f of (4B+b4, u, r)
                cT = loads.tile([BG * U, 256], F32, tag="cT")
                nc.vector.transpose(cT[:], c_pad[:])
                hT = loads.tile([BG * U, 256], F32, tag="hT")
                nc.vector.transpose(hT[:], h_pad[:])

                # ---- C path: u-sum then broadcast over u ------------------
                c_red = loads.tile([BG * U, 32], F32, tag="c_red")
                nc.vector.tensor_reduce(
                    c_red[:],
                    cT[:].rearrange("p (rb u) -> p rb u", u=U),
                    axis_x, op_add,
                )
                # c_red[32B+m, r*4 + b4] = c[4B+b4, r, m]
                xt_c = xt[32:48, :].rearrange(
                    "p (B b4 r u) -> p B r b4 u", B=4, b4=4, u=U
                )
                for B in range(4):
                    nc.vector.tensor_copy(
                        xt_c[:, B].bitcast(F32R),
                        c_red[32 * B : 32 * B + 16, :]
                        .rearrange("p (r b4) -> p r b4", b4=4)
                        .unsqueeze(3)
                        .broadcast_to((16, R, 4, U)),
                    )

                # ---- H path: per-B-band copies into xt --------------------
                xt_h = xt[64:80, :].rearrange(
                    "p (B b4 r u) -> p B r b4 u", B=4, b4=4, u=U
                )
                for B in range(4):
                    nc.vector.tensor_copy(
                        xt_h[:, B].bitcast(F32R),
                        hT[32 * B : 32 * B + 16, :].rearrange(
                            "p (r b4 u) -> p r b4 u", b4=4, u=U
                        ),
                    )

                # ---- F broadcast over u -----------------------------------
                nc.scalar.copy(
                    xt[0:16, :]
                    .rearrange("p (b r u) -> p b r u", r=R, u=U)
                    .bitcast(F32R),
                    ft[:, b0 : b0 + BG, :]
                    .unsqueeze(3)
                    .broadcast_to((FDIM, BG, R, U)),
                )

                # ---- MLP over two 512-row halves --------------------------
                for h in range(2):
                    cols = slice(h * TILE, (h + 1) * TILE)
                    xin = xt[:, cols].bitcast(F32R)

                    a1s = []
                    for s in range(4):
                        ps1 = p1p.tile([128, TILE], F32, tag="ps1")
                        nc.tensor.matmul(
                            ps1[:],
                            w1[:, s * 128 : (s + 1) * 128].bitcast(F32R),
                            xin,
                            start=True, stop=True,
                        )
                        a1_s = acts.tile([128, TILE], F32, tag=f"a1{s}")
                        # L1 bias folded in via the ones row -> plain relu
                        if s < 2:
                            nc.scalar.activation(
                                a1_s[:].bitcast(F32R), ps1[:], relu
                            )
                        else:
                            nc.vector.tensor_scalar_max(
                                a1_s[:].bitcast(F32R), ps1[:], 0.0
                            )
                        a1s.append(a1_s)

                    a2s = []
                    for t in range(4):
                        ps2 = p2p.tile([128, TILE], F32, tag="ps2")
                        for s in range(4):
                            nc.tensor.matmul(
                                ps2[:],
                                w2[:, s, t * 128 : (t + 1) * 128].bitcast(F32R),
                                a1s[s][:].bitcast(F32R),
                                start=(s == 0), stop=(s == 3),
                            )
                        a2_t = acts.tile([128, TILE], F32, tag=f"a2{t}")
                        if t < 3:
                            nc.scalar.activation(
                                a2_t[:].bitcast(F32R), ps2[:], relu,
                                bias=b2[:, t : t + 1],
                            )
                        else:
                            nc.vector.scalar_tensor_tensor(
                                a2_t[:].bitcast(F32R),
                                ps2[:],
                                b2[:, t : t + 1],
                                zero_bc[:, 0:1].broadcast_to((128, TILE)),
                                mybir.AluOpType.add,
                                mybir.AluOpType.max,
                            )
                        a2s.append(a2_t)

                    ps3 = psm.tile([M, TILE], F32, tag="sm")
                    for s in range(4):
                        nc.tensor.matmul(
                            ps3[:],
                            w3[:, s, :].bitcast(F32R),
                            a2s[s][:].bitcast(F32R),
                            start=(s == 0), stop=(s == 3),
                        )
                    yt = outs.tile([M, TILE], F32, tag="yt")
                    nc.scalar.activation(yt[:], ps3[:], tanh, bias=b3[:, 0:1])

                    # transpose back to row-major [512 x 16] and store
                    ps_o = psm.tile([128, 4, M], F32, tag="sm")
                    for q in range(4):
                        nc.tensor.matmul(
                            ps_o[:, q, :],
                            yt[:, q * 128 : (q + 1) * 128],
                            eye[:M, :M],
                            is_transpose=True, start=True, stop=True,
                        )
                    o_nat = outs.tile([128, 4, M], F32, tag="o_nat")
                    nc.vector.tensor_copy(o_nat[:], ps_o[:])
                    row0 = (2 * g + h) * TILE
                    nc.sync.dma_start(
                        out_rows[row0 : row0 + TILE].rearrange(
                            "(q p) m -> p q m", p=128
                        ),
                        o_nat[:],
                    )

    nc.compile()
    return nc


def _get_nc():
    if "nc" not in _CACHE:
        _CACHE["nc"] = _build()
    return _CACHE["nc"]


def _xinit():
    x = np.zeros((32, ROWS_CHUNK), dtype=np.float32)
    x[0, :] = 1.0
    return x


def run(inputs, trace=False):
    nc = _get_nc()
    np_in = {k: np.ascontiguousarray(np.asarray(v, dtype=np.float32))
             for k, v in inputs.items()}
    eye = np.eye(128, dtype=np.float32)
    xinit = _xinit()
    in_maps = []
    for i in range(N_CORES):
        sl = slice(i * B_SH, (i + 1) * B_SH)
        in_maps.append({
            "C": np_in["C"][sl],
            "F": np_in["F"][sl],
            "H": np_in["H"][sl],
            "W1": np_in["W1"], "b1": np_in["b1"],
            "W2": np_in["W2"], "b2": np_in["b2"],
            "W3": np_in["W3"], "b3": np_in["b3"],
            "eye128": eye,
            "xinit": xinit,
        })
    res = run_bass_kernel_spmd(nc, in_maps, list(range(N_CORES)), trace=trace)
    out = np.concatenate([res.results[i]["out"] for i in range(N_CORES)], axis=0)
    return out, res


def kernel(**inputs):
    out, _ = run(inputs, trace=False)
    return out


# revision 35
# speedup vs baseline: 1.1093x; 1.0106x over previous
"""Trainium2 Bass kernel for the Antenna message-generation MLP.

Reference computation (per batch b, RF-chain r, antenna u):
    x[b,r,u,:48] = concat(F[b,:,r], sum_u C[b,u,r,:], H[b,u,8r:8r+8], H[b,u,64+8r:64+8r+8])
    out[b,r,u,:] = tanh(relu(relu(x@W1+b1)@W2+b2)@W3+b3)

Strategy: pure data parallelism over the batch dim across 8 NeuronCores
(256 batches/core).  On each core the 16384 rows are processed in 32
tiles of 512 rows; activations are kept feature-on-partition so the
three matmul layers chain without transposes, all matmuls in float32r
(full-rate fp32).  The input gather (F transpose/broadcast, C u-sum,
H real/imag split) is built on-chip into a 64-partition X^T whose
layout folds the L1 bias in via a constant ones row:
    [0:16]=F  [16]=ones  [17:32]=0  [32:48]=c  [48:64]=0  [64:80]=h
with W1 zero-padded to match.  PE pair-packed transposes handle the
partition moves; DMA handles non-32-aligned partition relocation.
"""

import sys
import types

import numpy as np

# This image's `antenv` lacks `axon_hooks`; bass_utils imports it when
# BASS_TRACE is set.  Register a no-op stand-in so tracing degrades
# gracefully instead of crashing (real hook installed by test harness).
try:
    import antenv.axon_hooks  # noqa: F401
except ImportError:
    import antenv

    _m = types.ModuleType("antenv.axon_hooks")
    _m._hook = None
    _m.set_axon_ntff_profile_hook = lambda h: setattr(_m, "_hook", h)
    _m.get_axon_ntff_profile_hook = lambda: _m._hook
    sys.modules["antenv.axon_hooks"] = _m
    antenv.axon_hooks = _m

import concourse.bacc as bacc
import concourse.mybir as mybir
import concourse.tile as tile
from concourse.bass_utils import run_bass_kernel_spmd

F32 = mybir.dt.float32
F32R = mybir.dt.float32r

N_CORES = 8
B_FULL = 2048
B_SH = B_FULL // N_CORES    # 256 batches per core
U = 8
R = 8
M = 16
FDIM = 16
H1 = 512
H2 = 512

BG = 16                     # batches per build chunk
G = B_SH // BG              # 16 chunks per core
ROWS_CHUNK = BG * R * U     # 1024 rows per chunk
TILE = 512                  # rows per MLP tile (one PSUM bank of fp32)
XP = 80                     # X^T partitions (padded layout)

_CACHE = {}


def _build():
    nc = bacc.Bacc("TRN2", target_bir_lowering=False, debug=False)

    C_ext = nc.dram_tensor("C", [B_SH, U, R, M], F32, kind="ExternalInput")
    F_ext = nc.dram_tensor("F", [B_SH, FDIM, R], F32, kind="ExternalInput")
    H_ext = nc.dram_tensor("H", [B_SH, U, 2 * 64], F32, kind="ExternalInput")
    W1_ext = nc.dram_tensor("W1", [48, H1], F32, kind="ExternalInput")
    b1_ext = nc.dram_tensor("b1", [H1], F32, kind="ExternalInput")
    W2_ext = nc.dram_tensor("W2", [H1, H2], F32, kind="ExternalInput")
    b2_ext = nc.dram_tensor("b2", [H2], F32, kind="ExternalInput")
    W3_ext = nc.dram_tensor("W3", [H2, M], F32, kind="ExternalInput")
    b3_ext = nc.dram_tensor("b3", [M], F32, kind="ExternalInput")
    eye_ext = nc.dram_tensor("eye128", [128, 128], F32, kind="ExternalInput")
    # xinit row 0 is ones (the folded-bias row), rest zeros
    xinit_ext = nc.dram_tensor("xinit", [32, ROWS_CHUNK], F32, kind="ExternalInput")
    out_ext = nc.dram_tensor("out", [B_SH, R, U, M], F32, kind="ExternalOutput")

    out_rows = out_ext.ap().rearrange("b r u m -> (b r u) m")  # [16384, 16]

    relu = mybir.ActivationFunctionType.Relu
    tanh = mybir.ActivationFunctionType.Tanh
    axis_x = mybir.AxisListType.X
    op_add = mybir.AluOpType.add

    with tile.TileContext(nc) as tc:
        with (
            tc.tile_pool(name="consts", bufs=1) as consts,
            tc.tile_pool(name="loads", bufs=3) as loads,
            tc.tile_pool(name="acts", bufs=2) as acts,
            tc.tile_pool(name="outs", bufs=3) as outs,
            tc.tile_pool(name="p1", bufs=3, space="PSUM") as p1p,
            tc.tile_pool(name="p2", bufs=3, space="PSUM") as p2p,
            tc.tile_pool(name="psm", bufs=2, space="PSUM") as psm,
        ):
            # ---- constants -------------------------------------------------
            eye = consts.tile([128, 128], F32)
            nc.sync.dma_start(eye[:].bitcast(F32R), eye_ext.ap().bitcast(F32R))
            # W1 padded to the X^T layout, bias folded in as row 16
            w1raw = consts.tile([XP, H1], F32)
            nc.gpsimd.memset(w1raw[:], 0.0)
            nc.sync.dma_start(w1raw[0:16, :], W1_ext[0:16])
            nc.sync.dma_start(
                w1raw[16:17, :], b1_ext.ap().rearrange("(o n) -> o n", o=1)
            )
            nc.sync.dma_start(w1raw[32:48, :], W1_ext[16:32])
            nc.sync.dma_start(w1raw[64:80, :], W1_ext[32:48])
            w1 = consts.tile([XP, H1], F32)
            nc.vector.tensor_copy(w1[:].bitcast(F32R), w1raw[:])
            w2 = consts.tile([128, 4, H2], F32)
            nc.sync.dma_start(
                w2[:].bitcast(F32R),
                W2_ext.ap().rearrange("(s p) n -> p s n", p=128).bitcast(F32R),
            )
            w3 = consts.tile([128, 4, M], F32)
            nc.sync.dma_start(
                w3[:].bitcast(F32R),
                W3_ext.ap().rearrange("(s p) m -> p s m", p=128).bitcast(F32R),
            )
            b2 = consts.tile([128, 4], F32)
            nc.sync.dma_start(b2[:], b2_ext.ap().rearrange("(s p) -> p s", p=128))
            zero_bc = consts.tile([128, 1], F32)
            nc.gpsimd.memset(zero_bc[:], 0.0)
            b3 = consts.tile([M, 1], F32)
            nc.sync.dma_start(b3[:], b3_ext.ap().rearrange("(m o) -> m o", o=1))
            # F transposed once for the whole core: [16 x b x r]
            ft = consts.tile([FDIM, B_SH, R], F32)
            nc.sync.dma_start(ft[:], F_ext.ap().rearrange("b f r -> f b r"))

            # Persistent double-buffered X^T; one-time init of the ones/zero
            # band [16:32] (DMA may write any partition base).
            xts = []
            for i in range(2):
                xt = consts.tile([XP, ROWS_CHUNK], F32, tag=f"xt{i}")
                nc.sync.dma_start(
                    xt[16:32, :].bitcast(F32R), xinit_ext[0:16].bitcast(F32R)
                )
                nc.sync.dma_start(
                    xt[48:64, :].bitcast(F32R), xinit_ext[16:32].bitcast(F32R)
                )
                xts.append(xt)

            for g in range(G):
                b0 = g * BG
                xt = xts[g % 2]
                # ---- load chunk into r-padded 32-col blocks ---------------
                # c_pad [128 x 256]: cols r*32 + (m | 16 pad)
                # h_pad [128 x 256]: cols r*32 + (i*8+k | 16 pad)
                c_pad = loads.tile([BG * U, 256], F32, tag="c_pad")
                nc.gpsimd.memset(c_pad[:], 0.0)
                cp_v = c_pad[:].rearrange("p (r w) -> p r w", r=R)
                nc.sync.dma_start(
                    cp_v[:, :, 0:M],
                    C_ext[b0 : b0 + BG].rearrange("b u r m -> (b u) r m"),
                )
                h_pad = loads.tile([BG * U, 256], F32, tag="h_pad")
                nc.gpsimd.memset(h_pad[:], 0.0)
                hp_v = h_pad[:].rearrange("p (r w) -> p r w", r=R)
                h_src = H_ext[b0 : b0 + BG].rearrange(
                    "b u (i r k) -> (b u) i r k", i=2, r=R
                )
                for i in range(2):
                    nc.sync.dma_start(hp_v[:, :, 8 * i : 8 * i + 8], h_src[:, i])

                # ---- DVE 32x32 stream transposes --------------------------
                # cT[32B+m, r*32 + b4*8 + u] = C[4B+b4, u, r, m]   (m < 16)
                # hT[32B+f, r*32 + b4*8 + u] = H-feat f of (4B+b4, u, r)
                cT = loads.tile([BG * U, 256], F32, tag="cT")
                nc.vector.transpose(cT[:], c_pad[:])
                hT = loads.tile([BG * U, 256], F32, tag="hT")
                nc.vector.transpose(hT[:], h_pad[:])

                # ---- C path: u-sum then broadcast over u ------------------
                c_red = loads.tile([BG * U, 32], F32, tag="c_red")
                nc.vector.tensor_reduce(
                    c_red[:],
                    cT[:].rearrange("p (rb u) -> p rb u", u=U),
                    axis_x, op_add,
                )
                # c_red[32B+m, r*4 + b4] = c[4B+b4, r, m]
                xt_c = xt[32:48, :].rearrange(
                    "p (B b4 r u) -> p B r b4 u", B=4, b4=4, u=U
                )
                for B in range(4):
                    nc.vector.tensor_copy(
                        xt_c[:, B].bitcast(F32R),
                        c_red[32 * B : 32 * B + 16, :]
                        .rearrange("p (r b4) -> p r b4", b4=4)
                        .unsqueeze(3)
                        .broadcast_to((16, R, 4, U)),
                    )

                # ---- H path: per-B-band copies into xt --------------------
                xt_h = xt[64:80, :].rearrange(
                    "p (B b4 r u) -> p B r b4 u", B=4, b4=4, u=U
                )
                for B in range(4):
                    nc.vector.tensor_copy(
                        xt_h[:, B].bitcast(F32R),
                        hT[32 * B : 32 * B + 16, :].rearrange(
                            "p (r b4 u) -> p r b4 u", b4=4, u=U
                        ),
                    )

                # ---- F broadcast over u -----------------------------------
                nc.scalar.copy(
                    xt[0:16, :]
                    .rearrange("p (b r u) -> p b r u", r=R, u=U)
                    .bitcast(F32R),
                    ft[:, b0 : b0 + BG, :]
                    .unsqueeze(3)
                    .broadcast_to((FDIM, BG, R, U)),
                )

                # ---- MLP over two 512-row halves --------------------------
                for h in range(2):
                    cols = slice(h * TILE, (h + 1) * TILE)
                    xin = xt[:, cols].bitcast(F32R)

                    a1s = []
                    for s in range(4):
                        ps1 = p1p.tile([128, TILE], F32, tag="ps1")
                        nc.tensor.matmul(
                            ps1[:],
                            w1[:, s * 128 : (s + 1) * 128].bitcast(F32R),
                            xin,
                            start=True, stop=True,
                        )
                        a1_s = acts.tile([128, TILE], F32, tag=f"a1{s}")
                        # L1 bias folded in via the ones row -> plain relu
                        if s < 2:
                            nc.scalar.activation(
                                a1_s[:].bitcast(F32R), ps1[:], relu
                            )
                        else:
                            nc.vector.tensor_scalar_max(
                                a1_s[:].bitcast(F32R), ps1[:], 0.0
                            )
                        a1s.append(a1_s)

                    a2s = []
                    for t in range(4):
                        ps2 = p2p.tile([128, TILE], F32, tag="ps2")
                        for s in range(4):
                            nc.tensor.matmul(
                                ps2[:],
                                w2[:, s, t * 128 : (t + 1) * 128].bitcast(F32R),
                                a1s[s][:].bitcast(F32R),
                                start=(s == 0), stop=(s == 3),
                            )
                        a2_t = acts.tile([128, TILE], F32, tag=f"a2{t}")
                        nc.scalar.activation(
                            a2_t[:].bitcast(F32R), ps2[:], relu,
                            bias=b2[:, t : t + 1],
                        )
                        a2s.append(a2_t)

                    ps3 = psm.tile([M, TILE], F32, tag="sm")
                    for s in range(4):
                        nc.tensor.matmul(
                            ps3[:],
                            w3[:, s, :].bitcast(F32R),
                            a2s[s][:].bitcast(F32R),
                            start=(s == 0), stop=(s == 3),
                        )
                    yt = outs.tile([M, TILE], F32, tag="yt")
                    nc.scalar.activation(yt[:], ps3[:], tanh, bias=b3[:, 0:1])

                    # transpose back to row-major [512 x 16] and store
                    ps_o = psm.tile([128, 4, M], F32, tag="sm")
                    for q in range(4):
                        nc.tensor.matmul(
                            ps_o[:, q, :],
                            yt[:, q * 128 : (q + 1) * 128],
                            eye[:M, :M],
                            is_transpose=True, start=True, stop=True,
                        )
                    o_nat = outs.tile([128, 4, M], F32, tag="o_nat")
                    nc.vector.tensor_copy(o_nat[:], ps_o[:])
                    row0 = (2 * g + h) * TILE
                    nc.sync.dma_start(
                        out_rows[row0 : row0 + TILE].rearrange(
                            "(q p) m -> p q m", p=128
                        ),
                        o_nat[:],
                    )

    nc.compile()
    return nc


def _get_nc():
    if "nc" not in _CACHE:
        _CACHE["nc"] = _build()
    return _CACHE["nc"]


def _xinit():
    x = np.zeros((32, ROWS_CHUNK), dtype=np.float32)
    x[0, :] = 1.0
    return x


def run(inputs, trace=False):
    nc = _get_nc()
    np_in = {k: np.ascontiguousarray(np.asarray(v, dtype=np.float32))
             for k, v in inputs.items()}
    eye = np.eye(128, dtype=np.float32)
    xinit = _xinit()
    in_maps = []
    for i in range(N_CORES):
        sl = slice(i * B_SH, (i + 1) * B_SH)
        in_maps.append({
            "C": np_in["C"][sl],
            "F": np_in["F"][sl],
            "H": np_in["H"][sl],
            "W1": np_in["W1"], "b1": np_in["b1"],
            "W2": np_in["W2"], "b2": np_in["b2"],
            "W3": np_in["W3"], "b3": np_in["b3"],
            "eye128": eye,
            "xinit": xinit,
        })
    res = run_bass_kernel_spmd(nc, in_maps, list(range(N_CORES)), trace=trace)
    out = np.concatenate([res.results[i]["out"] for i in range(N_CORES)], axis=0)
    return out, res


def kernel(**inputs):
    out, _ = run(inputs, trace=False)
    return out


# revision 36
# speedup vs baseline: 1.1712x; 1.0558x over previous
"""Trainium2 Bass kernel for the Antenna message-generation MLP.

Reference computation (per batch b, RF-chain r, antenna u):
    x[b,r,u,:48] = concat(F[b,:,r], sum_u C[b,u,r,:], H[b,u,8r:8r+8], H[b,u,64+8r:64+8r+8])
    out[b,r,u,:] = tanh(relu(relu(x@W1+b1)@W2+b2)@W3+b3)

Strategy: pure data parallelism over the batch dim across 8 NeuronCores
(256 batches/core).  On each core the 16384 rows are processed in 32
tiles of 512 rows; activations are kept feature-on-partition so the
three matmul layers chain without transposes, all matmuls in float32r
(full-rate fp32).  The input gather (F transpose/broadcast, C u-sum,
H real/imag split) is built on-chip into a 64-partition X^T whose
layout folds the L1 bias in via a constant ones row:
    [0:16]=F  [16]=ones  [17:32]=0  [32:48]=c  [48:64]=0  [64:80]=h
with W1 zero-padded to match.  PE pair-packed transposes handle the
partition moves; DMA handles non-32-aligned partition relocation.
"""

import sys
import types

import numpy as np

# This image's `antenv` lacks `axon_hooks`; bass_utils imports it when
# BASS_TRACE is set.  Register a no-op stand-in so tracing degrades
# gracefully instead of crashing (real hook installed by test harness).
try:
    import antenv.axon_hooks  # noqa: F401
except ImportError:
    import antenv

    _m = types.ModuleType("antenv.axon_hooks")
    _m._hook = None
    _m.set_axon_ntff_profile_hook = lambda h: setattr(_m, "_hook", h)
    _m.get_axon_ntff_profile_hook = lambda: _m._hook
    sys.modules["antenv.axon_hooks"] = _m
    antenv.axon_hooks = _m

import concourse.bacc as bacc
import concourse.mybir as mybir
import concourse.tile as tile
from concourse.bass_utils import run_bass_kernel_spmd

F32 = mybir.dt.float32
F32R = mybir.dt.float32r
F16 = mybir.dt.float16

N_CORES = 8
B_FULL = 2048
B_SH = B_FULL // N_CORES    # 256 batches per core
U = 8
R = 8
M = 16
FDIM = 16
H1 = 512
H2 = 512

BG = 16                     # batches per build chunk
G = B_SH // BG              # 16 chunks per core
ROWS_CHUNK = BG * R * U     # 1024 rows per chunk
TILE = 512                  # rows per MLP tile (one PSUM bank of fp32)
XP = 80                     # X^T partitions (padded layout)

_CACHE = {}


def _build():
    nc = bacc.Bacc("TRN2", target_bir_lowering=False, debug=False)

    C_ext = nc.dram_tensor("C", [B_SH, U, R, M], F32, kind="ExternalInput")
    F_ext = nc.dram_tensor("F", [B_SH, FDIM, R], F32, kind="ExternalInput")
    H_ext = nc.dram_tensor("H", [B_SH, U, 2 * 64], F32, kind="ExternalInput")
    W1_ext = nc.dram_tensor("W1", [48, H1], F32, kind="ExternalInput")
    b1_ext = nc.dram_tensor("b1", [H1], F32, kind="ExternalInput")
    W2_ext = nc.dram_tensor("W2", [H1, H2], F32, kind="ExternalInput")
    b2_ext = nc.dram_tensor("b2", [H2], F32, kind="ExternalInput")
    W3_ext = nc.dram_tensor("W3", [H2, M], F32, kind="ExternalInput")
    b3_ext = nc.dram_tensor("b3", [M], F32, kind="ExternalInput")
    eye_ext = nc.dram_tensor("eye128", [128, 128], F32, kind="ExternalInput")
    # xinit row 0 is ones (the folded-bias row), rest zeros
    xinit_ext = nc.dram_tensor("xinit", [32, ROWS_CHUNK], F32, kind="ExternalInput")
    out_ext = nc.dram_tensor("out", [B_SH, R, U, M], F32, kind="ExternalOutput")

    out_rows = out_ext.ap().rearrange("b r u m -> (b r u) m")  # [16384, 16]

    relu = mybir.ActivationFunctionType.Relu
    tanh = mybir.ActivationFunctionType.Tanh
    axis_x = mybir.AxisListType.X
    op_add = mybir.AluOpType.add

    with tile.TileContext(nc) as tc:
        with (
            tc.tile_pool(name="consts", bufs=1) as consts,
            tc.tile_pool(name="loads", bufs=3) as loads,
            tc.tile_pool(name="acts", bufs=2) as acts,
            tc.tile_pool(name="outs", bufs=3) as outs,
            tc.tile_pool(name="p1", bufs=3, space="PSUM") as p1p,
            tc.tile_pool(name="p2", bufs=3, space="PSUM") as p2p,
            tc.tile_pool(name="psm", bufs=2, space="PSUM") as psm,
        ):
            # ---- constants -------------------------------------------------
            eye = consts.tile([128, 128], F32)
            nc.sync.dma_start(eye[:].bitcast(F32R), eye_ext.ap().bitcast(F32R))
            # W1 padded to the X^T layout, bias folded in as row 16
            w1raw = consts.tile([XP, H1], F32)
            nc.gpsimd.memset(w1raw[:], 0.0)
            nc.sync.dma_start(w1raw[0:16, :], W1_ext[0:16])
            nc.sync.dma_start(
                w1raw[16:17, :], b1_ext.ap().rearrange("(o n) -> o n", o=1)
            )
            nc.sync.dma_start(w1raw[32:48, :], W1_ext[16:32])
            nc.sync.dma_start(w1raw[64:80, :], W1_ext[32:48])
            w1 = consts.tile([XP, H1], F32)
            nc.vector.tensor_copy(w1[:].bitcast(F32R), w1raw[:])
            w2 = consts.tile([128, 4, H2], F16)
            nc.gpsimd.dma_start(
                w2[:], W2_ext.ap().rearrange("(s p) n -> p s n", p=128)
            )
            w3 = consts.tile([128, 4, M], F32)
            nc.sync.dma_start(
                w3[:].bitcast(F32R),
                W3_ext.ap().rearrange("(s p) m -> p s m", p=128).bitcast(F32R),
            )
            b2 = consts.tile([128, 4], F32)
            nc.sync.dma_start(b2[:], b2_ext.ap().rearrange("(s p) -> p s", p=128))
            zero_bc = consts.tile([128, 1], F32)
            nc.gpsimd.memset(zero_bc[:], 0.0)
            b3 = consts.tile([M, 1], F32)
            nc.sync.dma_start(b3[:], b3_ext.ap().rearrange("(m o) -> m o", o=1))
            # F transposed once for the whole core: [16 x b x r]
            ft = consts.tile([FDIM, B_SH, R], F32)
            nc.sync.dma_start(ft[:], F_ext.ap().rearrange("b f r -> f b r"))

            # Persistent double-buffered X^T; one-time init of the ones/zero
            # band [16:32] (DMA may write any partition base).
            xts = []
            for i in range(2):
                xt = consts.tile([XP, ROWS_CHUNK], F32, tag=f"xt{i}")
                nc.sync.dma_start(
                    xt[16:32, :].bitcast(F32R), xinit_ext[0:16].bitcast(F32R)
                )
                nc.sync.dma_start(
                    xt[48:64, :].bitcast(F32R), xinit_ext[16:32].bitcast(F32R)
                )
                xts.append(xt)

            for g in range(G):
                b0 = g * BG
                xt = xts[g % 2]
                # ---- load chunk into r-padded 32-col blocks ---------------
                # c_pad [128 x 256]: cols r*32 + (m | 16 pad)
                # h_pad [128 x 256]: cols r*32 + (i*8+k | 16 pad)
                c_pad = loads.tile([BG * U, 256], F32, tag="c_pad")
                nc.gpsimd.memset(c_pad[:], 0.0)
                cp_v = c_pad[:].rearrange("p (r w) -> p r w", r=R)
                nc.sync.dma_start(
                    cp_v[:, :, 0:M],
                    C_ext[b0 : b0 + BG].rearrange("b u r m -> (b u) r m"),
                )
                h_pad = loads.tile([BG * U, 256], F32, tag="h_pad")
                nc.gpsimd.memset(h_pad[:], 0.0)
                hp_v = h_pad[:].rearrange("p (r w) -> p r w", r=R)
                h_src = H_ext[b0 : b0 + BG].rearrange(
                    "b u (i r k) -> (b u) i r k", i=2, r=R
                )
                for i in range(2):
                    nc.sync.dma_start(hp_v[:, :, 8 * i : 8 * i + 8], h_src[:, i])

                # ---- DVE 32x32 stream transposes --------------------------
                # cT[32B+m, r*32 + b4*8 + u] = C[4B+b4, u, r, m]   (m < 16)
                # hT[32B+f, r*32 + b4*8 + u] = H-feat f of (4B+b4, u, r)
                cT = loads.tile([BG * U, 256], F32, tag="cT")
                nc.vector.transpose(cT[:], c_pad[:])
                hT = loads.tile([BG * U, 256], F32, tag="hT")
                nc.vector.transpose(hT[:], h_pad[:])

                # ---- C path: u-sum then broadcast over u ------------------
                c_red = loads.tile([BG * U, 32], F32, tag="c_red")
                nc.vector.tensor_reduce(
                    c_red[:],
                    cT[:].rearrange("p (rb u) -> p rb u", u=U),
                    axis_x, op_add,
                )
                # c_red[32B+m, r*4 + b4] = c[4B+b4, r, m]
                xt_c = xt[32:48, :].rearrange(
                    "p (B b4 r u) -> p B r b4 u", B=4, b4=4, u=U
                )
                for B in range(4):
                    nc.vector.tensor_copy(
                        xt_c[:, B].bitcast(F32R),
                        c_red[32 * B : 32 * B + 16, :]
                        .rearrange("p (r b4) -> p r b4", b4=4)
                        .unsqueeze(3)
                        .broadcast_to((16, R, 4, U)),
                    )

                # ---- H path: per-B-band copies into xt --------------------
                xt_h = xt[64:80, :].rearrange(
                    "p (B b4 r u) -> p B r b4 u", B=4, b4=4, u=U
                )
                for B in range(4):
                    nc.vector.tensor_copy(
                        xt_h[:, B].bitcast(F32R),
                        hT[32 * B : 32 * B + 16, :].rearrange(
                            "p (r b4 u) -> p r b4 u", b4=4, u=U
                        ),
                    )

                # ---- F broadcast over u -----------------------------------
                nc.scalar.copy(
                    xt[0:16, :]
                    .rearrange("p (b r u) -> p b r u", r=R, u=U)
                    .bitcast(F32R),
                    ft[:, b0 : b0 + BG, :]
                    .unsqueeze(3)
                    .broadcast_to((FDIM, BG, R, U)),
                )

                # ---- MLP over two 512-row halves --------------------------
                for h in range(2):
                    cols = slice(h * TILE, (h + 1) * TILE)
                    xin = xt[:, cols].bitcast(F32R)

                    a1s = []
                    for s in range(4):
                        ps1 = p1p.tile([128, TILE], F32, tag="ps1")
                        nc.tensor.matmul(
                            ps1[:],
                            w1[:, s * 128 : (s + 1) * 128].bitcast(F32R),
                            xin,
                            start=True, stop=True,
                        )
                        a1_s = acts.tile([128, TILE], F16, tag=f"a1{s}")
                        # L1 bias folded in via the ones row -> plain relu
                        if s < 2:
                            nc.scalar.activation(a1_s[:], ps1[:], relu)
                        else:
                            nc.vector.tensor_scalar_max(a1_s[:], ps1[:], 0.0)
                        a1s.append(a1_s)

                    a2s = []
                    for t in range(4):
                        ps2 = p2p.tile([128, TILE], F32, tag="ps2")
                        for s in range(4):
                            nc.tensor.matmul(
                                ps2[:],
                                w2[:, s, t * 128 : (t + 1) * 128],
                                a1s[s][:],
                                start=(s == 0), stop=(s == 3),
                            )
                        a2_t = acts.tile([128, TILE], F32, tag=f"a2{t}")
                        nc.scalar.activation(
                            a2_t[:].bitcast(F32R), ps2[:], relu,
                            bias=b2[:, t : t + 1],
                        )
                        a2s.append(a2_t)

                    ps3 = psm.tile([M, TILE], F32, tag="sm")
                    for s in range(4):
                        nc.tensor.matmul(
                            ps3[:],
                            w3[:, s, :].bitcast(F32R),
                            a2s[s][:].bitcast(F32R),
                            start=(s == 0), stop=(s == 3),
                        )
                    yt = outs.tile([M, TILE], F32, tag="yt")
                    nc.scalar.activation(yt[:], ps3[:], tanh, bias=b3[:, 0:1])

                    # transpose back to row-major [512 x 16] and store
                    ps_o = psm.tile([128, 4, M], F32, tag="sm")
                    for q in range(4):
                        nc.tensor.matmul(
                            ps_o[:, q, :],
                            yt[:, q * 128 : (q + 1) * 128],
                            eye[:M, :M],
                            is_transpose=True, start=True, stop=True,
                        )
                    o_nat = outs.tile([128, 4, M], F32, tag="o_nat")
                    nc.vector.tensor_copy(o_nat[:], ps_o[:])
                    row0 = (2 * g + h) * TILE
                    nc.sync.dma_start(
                        out_rows[row0 : row0 + TILE].rearrange(
                            "(q p) m -> p q m", p=128
                        ),
                        o_nat[:],
                    )

    nc.compile()
    return nc


def _get_nc():
    if "nc" not in _CACHE:
        _CACHE["nc"] = _build()
    return _CACHE["nc"]


def _xinit():
    x = np.zeros((32, ROWS_CHUNK), dtype=np.float32)
    x[0, :] = 1.0
    return x


def run(inputs, trace=False):
    nc = _get_nc()
    np_in = {k: np.ascontiguousarray(np.asarray(v, dtype=np.float32))
             for k, v in inputs.items()}
    eye = np.eye(128, dtype=np.float32)
    xinit = _xinit()
    in_maps = []
    for i in range(N_CORES):
        sl = slice(i * B_SH, (i + 1) * B_SH)
        in_maps.append({
            "C": np_in["C"][sl],
            "F": np_in["F"][sl],
            "H": np_in["H"][sl],
            "W1": np_in["W1"], "b1": np_in["b1"],
            "W2": np_in["W2"], "b2": np_in["b2"],
            "W3": np_in["W3"], "b3": np_in["b3"],
            "eye128": eye,
            "xinit": xinit,
        })
    res = run_bass_kernel_spmd(nc, in_maps, list(range(N_CORES)), trace=trace)
    out = np.concatenate([res.results[i]["out"] for i in range(N_CORES)], axis=0)
    return out, res


def kernel(**inputs):
    out, _ = run(inputs, trace=False)
    return out
